# revision 1
# baseline (speedup 1.0000x reference)
"""Chamfer distance loss on 8 Trainium2 NeuronCores.

Problem: prediction [4, 8192, 3], target [4, 8192, 3] (f32).
  d2[b,n,m] = ||pred[b,n] - tgt[b,m]||^2  (clamped at 0)
  out = mean_{b,n} min_m d2  +  mean_{b,m} min_n d2     (scalar f32)

Sharding: 8 cores = 4 batches x 2 halves of the N axis. Each core computes
its 4096 x 8192 block of the distance matrix flash-style (never
materialized in DRAM):

  - d2 tiles are produced by a single K=24 bf16 matmul per [128,512] tile
    using the augmented-vector trick  d2 = 1*y2 + x2*1 + sum_i (-2 x_i)*y_i
    with every f32 factor split into 3 bf16 limbs (hi/mid/lo) so products
    are exact to ~2^-26 while the PE runs at full bf16 speed (fp32 matmul
    is 4x slower on the PE).
  - ScalarE drains PSUM -> SBUF converting to f16 (values, not operands,
    so rounding is relative: ~2^-11).
  - VectorE keeps a running per-column min (elementwise f16 min, 2x mode)
    and computes each row min with one tensor_tensor_scan(min,min) over the
    two row halves (the last scan column is the full row min). The fused
    reduce ops tensor_tensor_reduce / tensor_scalar+accum_out crash the HW
    exec unit in this environment; the scan and plain tensor_tensor /
    tensor_reduce are HW-verified here.
  - Column mins are folded over the partition axis on-device (PE
    transpose + DVE reduce) so outputs are tiny (axon tunnel is slow).

The paired 24-row operand matrices are host-assembled and shipped as one
flat bf16 array per core (the axon tunnel charges ~100ms per staged
array, so fewer/larger arrays win).

Host combines: per-batch row mins (exact concat) + per-column partial
mins (elementwise min of the two half-N cores), relu, means.
"""

import sys

if "/opt/trn_rl_repo" not in sys.path:
    sys.path.insert(0, "/opt/trn_rl_repo")

import numpy as np
import ml_dtypes


def _install_neff_cache():
    """Cache compiled NEFFs on disk keyed by BIR hash.

    The stock bass_exec path recompiles (~100s of walrus) in every fresh
    process; the program here is deterministic, so a byte-keyed cache is
    safe and makes repeat runs start in seconds.
    """
    import hashlib
    import os
    import shutil

    from concourse import bass2jax as _b2j
    from concourse import bass_utils as _bu

    if getattr(_bu, "_chamfer_neff_cache", False):
        return
    orig = _bu.compile_bir_kernel

    def _key(bir_json):
        # The BIR bytes embed debug info with kernel.py's load path, which
        # varies by directory, so a content hash is not portable. Only this
        # module's (deterministic) program flows through the patched
        # compile in a kernel.py process, so key on a version constant —
        # bump it on any _build_program change.
        return "chamfer-v5-k24-scan128"

    def cached(bir_json, tmpdir, neff_name="file.neff"):
        key = _key(bir_json)
        cdir = os.environ.get("CHAMFER_NEFF_CACHE", "/tmp/chamfer_neff_cache")
        cpath = os.path.join(cdir, key + ".neff")
        out = os.path.join(tmpdir, neff_name)
        try:
            if os.path.exists(cpath):
                shutil.copyfile(cpath, out)
                return out
        except OSError:
            pass
        p = orig(bir_json, tmpdir, neff_name)
        try:
            os.makedirs(cdir, exist_ok=True)
            tmp = cpath + f".tmp{os.getpid()}"
            shutil.copyfile(p, tmp)
            os.replace(tmp, cpath)
        except OSError:
            pass
        return p

    _bu.compile_bir_kernel = cached
    _b2j.compile_bir_kernel = cached
    _bu._chamfer_neff_cache = True


_install_neff_cache()

B, N, M, D = 4, 8192, 8192, 3
N_CORES = 8
NH = N // 2          # rows per core (4096)
P = 128              # partitions
NT = NH // P         # n-tiles per core (32)
K = 24               # contraction rows of the split-bf16 augmented matmul
BIG = 60000.0        # > max possible d2 (~80), fits in f16
XY_LEN = K * NH + K * M  # paired x rows then paired y rows, flat

import os as _os

Z_BUFS = int(_os.environ.get("CHAMFER_Z_BUFS", "2"))
FOLD_STOP = int(_os.environ.get("CHAMFER_FOLD_STOP", "128"))
ROWMIN_MODE = _os.environ.get("CHAMFER_ROWMIN", "scan")  # fold | scan

# Pairing of the 24 product rows: (x source, y source) where sources index
# the 12 "unique" limb rows per side, or "ones" for the constant row.
# x-unique rows: [x2_0, x2_1, x2_2, a00,a01,a02, a10,a11,a12, a20,a21,a22]
# y-unique rows: [y2_0, y2_1, y2_2, b00,b01,b02, b10,b11,b12, b20,b21,b22]
# where a_i* = limbs of -2*x_i and b_i* = limbs of y_i.
PAIRS = (
    [("ones", 0), ("ones", 1), ("ones", 2), (0, "ones"), (1, "ones"), (2, "ones")]
    + [
        (3 + 3 * i + dx, 3 + 3 * i + dy)
        for i in range(3)
        for dx, dy in ((0, 0), (0, 1), (1, 0), (0, 2), (2, 0), (1, 1))
    ]
)
assert len(PAIRS) == K

# Set by test.py.
TRACE = False
LAST_RESULTS = None

_PROGRAM = None


def _build_program():
    from concourse import bacc, tile
    import concourse.mybir as mybir

    f32 = mybir.dt.float32
    f16 = mybir.dt.float16
    bf16 = mybir.dt.bfloat16

    nc = bacc.Bacc(
        "TRN2",
        target_bir_lowering=False,
        debug=False,
        enable_asserts=False,
    )

    xy_d = nc.dram_tensor("xy", [XY_LEN], bf16, kind="ExternalInput").ap()
    rowmin_d = nc.dram_tensor("rowmin", [P, NT], f32, kind="ExternalOutput").ap()
    # colmin[q, k] = min_p over partitions of column m = 128*k + q
    colmin_d = nc.dram_tensor("colmin", [P, M // P], f32, kind="ExternalOutput").ap()

    xh_d = xy_d[0 : K * NH].rearrange("(k n) -> k n", k=K)
    yh_d = xy_d[K * NH :].rearrange("(k n) -> k n", k=K)

    with tile.TileContext(nc) as tc:
        from contextlib import ExitStack

        with ExitStack() as ctx:
            const_pool = ctx.enter_context(tc.tile_pool(name="const", bufs=1))
            z_pool = ctx.enter_context(tc.tile_pool(name="z", bufs=Z_BUFS))
            psum_pool = ctx.enter_context(
                tc.tile_pool(name="psum", bufs=2, space="PSUM")
            )
            acc_pool = ctx.enter_context(tc.tile_pool(name="acc", bufs=1))

            # paired operand matrices are host-assembled; two HWDGE queues
            xh = const_pool.tile([K, NH], bf16)
            yh = const_pool.tile([K, M], bf16)
            nc.sync.dma_start(xh[:], xh_d[:])
            nc.scalar.dma_start(yh[:, : M // 2], yh_d[:, : M // 2])
            nc.sync.dma_start(yh[:, M // 2 :], yh_d[:, M // 2 :])

            colacc = acc_pool.tile([P, M], f16)
            rowmin = acc_pool.tile([P, NT], f32)

            for t in range(NT):
                z = z_pool.tile([P, M], f16, tag="z")
                lhsT = xh[:, t * P : (t + 1) * P]
                for g in range(4):  # four PSUM groups of 4 banks each
                    ps = psum_pool.tile([P, 4 * 512], f32, tag="ps")
                    for j in range(4):
                        mm = g * 4 + j
                        nc.tensor.matmul(
                            ps[:, j * 512 : (j + 1) * 512],
                            lhsT,
                            yh[:, mm * 512 : (mm + 1) * 512],
                            start=True,
                            stop=True,
                        )
                    nc.scalar.activation(
                        z[:, g * 2048 : (g + 1) * 2048],
                        ps[:],
                        mybir.ActivationFunctionType.Copy,
                    )
                # running per-column min across n-tiles; the first tile just
                # initializes colacc (single-src f16 copy runs at 4x vs the
                # 2x tensor_tensor, and saves the memset)
                if t == 0:
                    nc.vector.tensor_copy(colacc[:], z[:])
                else:
                    nc.vector.tensor_tensor(
                        colacc[:], colacc[:], z[:], mybir.AluOpType.min
                    )
                # per-row min of this n-tile
                if ROWMIN_MODE == "scan":
                    # one running-min scan over both halves; last column is
                    # the full row min
                    sc = z_pool.tile([P, M // 2], f16, tag="scan")
                    nc.vector.tensor_tensor_scan(
                        sc[:],
                        z[:, : M // 2],
                        z[:, M // 2 :],
                        initial=BIG,
                        op0=mybir.AluOpType.min,
                        op1=mybir.AluOpType.min,
                    )
                    # tiny extract on the (slack) scalar engine
                    nc.scalar.copy(rowmin[:, t : t + 1], sc[:, M // 2 - 1 : M // 2])
                else:
                    # fold tree at 2x, small reduce
                    src = z
                    w = M
                    while w > FOLD_STOP:
                        w //= 2
                        nxt = z_pool.tile([P, w], f16, tag=f"fold{w}")
                        nc.vector.tensor_tensor(
                            nxt[:], src[:, :w], src[:, w : 2 * w], mybir.AluOpType.min
                        )
                        src = nxt
                    nc.vector.tensor_reduce(
                        rowmin[:, t : t + 1],
                        src[:],
                        axis=mybir.AxisListType.X,
                        op=mybir.AluOpType.min,
                    )

            nc.sync.dma_start(rowmin_d[:], rowmin[:])

            # --- column fold: min over the 128-partition axis of colacc ---
            ident = const_pool.tile([P, P], f16)
            rowidx = const_pool.tile([P, P], f16)
            colidx = const_pool.tile([P, P], f16)
            nc.gpsimd.iota(
                rowidx[:], [[0, P]], channel_multiplier=1,
                allow_small_or_imprecise_dtypes=True,
            )
            nc.gpsimd.iota(
                colidx[:], [[1, P]], channel_multiplier=0,
                allow_small_or_imprecise_dtypes=True,
            )
            nc.vector.tensor_tensor(
                ident[:], rowidx[:], colidx[:], mybir.AluOpType.is_equal
            )

            colmin = acc_pool.tile([P, M // P], f32)
            for half in range(2):
                tp = psum_pool.tile([P, 4096], f16, tag="ps")
                for k in range(32):
                    nc.tensor.transpose(
                        tp[:, k * P : (k + 1) * P],
                        colacc[:, (half * 32 + k) * P : (half * 32 + k + 1) * P],
                        ident[:],
                    )
                nc.vector.tensor_reduce(
                    colmin[:, half * 32 : (half + 1) * 32],
                    tp.rearrange("p (k q) -> p k q", q=P),
                    axis=mybir.AxisListType.X,
                    op=mybir.AluOpType.min,
                )
            nc.sync.dma_start(colmin_d[:], colmin[:])

    nc.compile()
    return nc


def _get_program():
    global _PROGRAM
    if _PROGRAM is None:
        _PROGRAM = _build_program()
    return _PROGRAM


def _split3(a):
    """Split float64 array into 3 bf16 limbs: a ~= l0 + l1 + l2."""
    l0 = a.astype(ml_dtypes.bfloat16)
    r = a - l0.astype(np.float64)
    l1 = r.astype(ml_dtypes.bfloat16)
    r = r - l1.astype(np.float64)
    l2 = r.astype(ml_dtypes.bfloat16)
    return l0, l1, l2


def _unique_rows(pts, negate_double):
    """12 unique bf16 limb rows for one side.

    pts: [L, 3] f64. Rows: 3 limbs of |p|^2, then 3 limbs of each
    coordinate (scaled by -2 when negate_double).
    """
    sq = (pts * pts).sum(1)
    rows = list(_split3(sq))
    scale = -2.0 if negate_double else 1.0
    for i in range(3):
        rows.extend(_split3(scale * pts[:, i]))
    return np.stack(rows).astype(ml_dtypes.bfloat16)


def _core_input(x, y):
    """Flat per-core input: paired x rows [24, NH] then paired y rows [24, M]."""
    xu = _unique_rows(x, negate_double=True)
    yu = _unique_rows(y, negate_double=False)
    ox = np.ones(x.shape[0], ml_dtypes.bfloat16)
    oy = np.ones(y.shape[0], ml_dtypes.bfloat16)
    xh = np.stack([ox if sx == "ones" else xu[sx] for sx, _ in PAIRS])
    yh = np.stack([oy if sy == "ones" else yu[sy] for _, sy in PAIRS])
    return np.concatenate([xh.ravel(), yh.ravel()])


def kernel(prediction, target):
    global LAST_RESULTS
    from concourse.bass_utils import run_bass_kernel_spmd

    nc = _get_program()

    pred = np.asarray(prediction, np.float64)
    tgt = np.asarray(target, np.float64)

    in_maps = []
    for c in range(N_CORES):
        b, h = divmod(c, 2)
        in_maps.append(
            {"xy": _core_input(pred[b, h * NH : (h + 1) * NH], tgt[b])}
        )

    res = run_bass_kernel_spmd(
        nc, in_maps, core_ids=list(range(N_CORES)), trace=TRACE
    )
    LAST_RESULTS = res

    cham_x = np.zeros(B)
    cham_y = np.zeros(B)
    for b in range(B):
        row = []
        cols = []
        for h in range(2):
            r = res.results[2 * b + h]
            # rowmin[p, t] is the min for row n = t*128 + p
            row.append(np.asarray(r["rowmin"], np.float64).T.ravel())
            # colmin[q, k] is the partial min for column m = k*128 + q
            cols.append(np.asarray(r["colmin"], np.float64).T.ravel())
        rowmin = np.concatenate(row)            # [N]
        colmin = np.minimum(cols[0], cols[1])   # [M]
        cham_x[b] = np.maximum(rowmin, 0.0).mean()
        cham_y[b] = np.maximum(colmin, 0.0).mean()

    return np.float32(cham_x.mean() + cham_y.mean())



# revision 26
# speedup vs baseline: 1.0139x; 1.0139x over previous
"""Chamfer distance loss on 8 Trainium2 NeuronCores.

Problem: prediction [4, 8192, 3], target [4, 8192, 3] (f32).
  d2[b,n,m] = ||pred[b,n] - tgt[b,m]||^2  (clamped at 0)
  out = mean_{b,n} min_m d2  +  mean_{b,m} min_n d2     (scalar f32)

Sharding: 8 cores = 4 batches x 2 halves of the N axis. Each core computes
its 4096 x 8192 block of the distance matrix flash-style (never
materialized in DRAM):

  - The PE produces s = -d2 via a single K=24 bf16 matmul per [128,512]
    tile using the augmented-vector trick  s = 1*(-y2) + (-x2)*1 +
    sum_i (2 x_i)*y_i, every f32 factor split into 3 bf16 limbs so
    products are exact to ~2^-26 at full bf16 PE speed. The NEGATION is
    free (host flips the limb signs) and turns both min reductions into
    MAX, which the gpsimd partition_all_reduce ISA op supports (used in
    the column-fold tail).
  - Per tile (4 PSUM groups of 2048 cols): Act drains all four groups to
    z [128,8192] f16; DVE runs the running column-max (one f16 2x
    tensor_tensor) and the row-max (one tensor_tensor_scan over the two
    row halves; its last column is the full row max).
    This Act/DVE split is FORCED: the walrus BIR verifier on this
    toolchain rejects tensor_tensor / tensor_tensor_scan / tensor_reduce
    / psum-reads on the Pool engine and 3-D tensor_tensor reads of PSUM,
    so the v6-v10 designs that balanced the drain+reductions across
    Act/DVE/Pool (~6.4us/tile in the cost model) cannot compile. DVE is
    the bound: ~8.65us/tile.
  - Scheduling details that remove the baseline's pipeline stalls
    (learned from TimelineSim traces): deps are per-engine COUNTING
    semaphores (a cross-engine wait serializes behind ALL earlier work on
    the source engine), so the row-max extraction lags TWO tiles in the
    Act stream, and input DMAs ship the tile-0-gating slices first.
  - Column-max fold over the partition axis at the end is split: PE
    transposes + DVE 3-D tensor_reduce for the first NB_D col-blocks
    (baseline-style, verifier-legal), gpsimd partition_all_reduce(max)
    for the rest, in parallel.

The paired 24-row operand matrices are host-assembled and shipped as one
flat bf16 array per core (the axon tunnel charges ~100ms per staged
array, so fewer/larger arrays win).

Host combines: row maxes (negate -> row mins), column maxes (elementwise
min of the two half-N cores after negation), relu, means.
"""

import sys

if "/opt/trn_rl_repo" not in sys.path:
    sys.path.insert(0, "/opt/trn_rl_repo")

import os as _os

import numpy as np
import ml_dtypes

B, N, M, D = 4, 8192, 8192, 3
N_CORES = 8
NH = N // 2          # rows per core (4096)
P = 128              # partitions
NT = NH // P         # n-tiles per core (32)
K = 24               # contraction rows of the split-bf16 augmented matmul
BIG = 60000.0        # > max possible d2 (~350), fits in f16
XY_LEN = K * NH + K * M  # paired x rows then paired y rows, flat

Z_BUFS = int(_os.environ.get("CHAMFER_Z_BUFS", "2"))
GB = 2048                                          # cols per PSUM group
NB_D = int(_os.environ.get("CHAMFER_NBD", "28"))  # col-blocks folded by DVE
POOL_OPS = int(_os.environ.get("CHAMFER_POOL", "1"))  # 0: all-DVE tail
EXT_LAG = 2                                        # row-max extract lag, tiles

_CFG = f"v12-{K}-{Z_BUFS}-{NB_D}-{POOL_OPS}"


def _install_neff_cache():
    """Cache compiled NEFFs on disk keyed by a config-versioned constant.

    The stock bass_exec path recompiles (~100s of walrus) in every fresh
    process; the program here is deterministic given _CFG, so a
    config-keyed cache is safe and makes repeat runs start in seconds.
    """
    import os
    import shutil

    from concourse import bass2jax as _b2j
    from concourse import bass_utils as _bu

    if getattr(_bu, "_chamfer_neff_cache", None) == _CFG:
        return
    orig = getattr(_bu, "_chamfer_orig_compile", None) or _bu.compile_bir_kernel
    _bu._chamfer_orig_compile = orig

    def cached(bir_json, tmpdir, neff_name="file.neff"):
        key = "chamfer-" + _CFG
        cdir = os.environ.get("CHAMFER_NEFF_CACHE", "/tmp/chamfer_neff_cache")
        cpath = os.path.join(cdir, key + ".neff")
        out = os.path.join(tmpdir, neff_name)
        try:
            if os.path.exists(cpath):
                shutil.copyfile(cpath, out)
                return out
        except OSError:
            pass
        p = orig(bir_json, tmpdir, neff_name)
        try:
            os.makedirs(cdir, exist_ok=True)
            tmp = cpath + f".tmp{os.getpid()}"
            shutil.copyfile(p, tmp)
            os.replace(tmp, cpath)
        except OSError:
            pass
        return p

    _bu.compile_bir_kernel = cached
    _b2j.compile_bir_kernel = cached
    _bu._chamfer_neff_cache = _CFG


_install_neff_cache()

# Pairing of the 24 product rows: (x source, y source) where sources index
# the 12 "unique" limb rows per side, or "ones" for the constant row.
# x-unique rows: [nx2_0, nx2_1, nx2_2, a00,a01,a02, a10,a11,a12, a20,a21,a22]
# y-unique rows: [ny2_0, ny2_1, ny2_2, b00,b01,b02, b10,b11,b12, b20,b21,b22]
# where nx2/ny2 = limbs of -|p|^2, a_i* = limbs of +2*x_i, b_i* = limbs
# of y_i  (so the matmul accumulates s = -d2).
PAIRS = (
    [("ones", 0), ("ones", 1), ("ones", 2), (0, "ones"), (1, "ones"), (2, "ones")]
    + [
        (3 + 3 * i + dx, 3 + 3 * i + dy)
        for i in range(3)
        for dx, dy in ((0, 0), (0, 1), (1, 0), (0, 2), (2, 0), (1, 1))
    ]
)
assert len(PAIRS) == K

# Set by test.py.
TRACE = False
LAST_RESULTS = None

_PROGRAM = None


def _build_program():
    from concourse import bacc, tile, bass_isa
    import concourse.mybir as mybir

    f32 = mybir.dt.float32
    f16 = mybir.dt.float16
    bf16 = mybir.dt.bfloat16
    mx = mybir.AluOpType.max

    nc = bacc.Bacc(
        "TRN2",
        target_bir_lowering=False,
        debug=False,
        enable_asserts=False,
    )

    xy_d = nc.dram_tensor("xy", [XY_LEN], bf16, kind="ExternalInput").ap()
    # out[:, 0:32] row maxes of s (rowmax[p,t] = max_m s for n = t*128+p);
    # [:, 32:35] extra row-max partials of tile 0 (its scan is split in 4
    # so DVE starts as each PSUM group drains instead of idling through
    # the whole first tile); [:, 35] extra partial of tile NT-1 (split in
    # 2 so the column-fold tail starts after its first half);
    # [:, 36:36+NB_D] colmax[q, k] = max_p colacc[p, 128k+q]
    out_d = nc.dram_tensor("out", [P, 36 + NB_D], f32, kind="ExternalOutput").ap()
    # colmax for columns [NB_D*128, 8192), via gpsimd partition_all_reduce
    WC = M - NB_D * P
    outc_d = nc.dram_tensor("outc", [1, WC], f16, kind="ExternalOutput").ap()

    xh_d = xy_d[0 : K * NH].rearrange("(k n) -> k n", k=K)
    yh_d = xy_d[K * NH :].rearrange("(k n) -> k n", k=K)

    with tile.TileContext(nc) as tc:
        from contextlib import ExitStack

        with ExitStack() as ctx:
            const_pool = ctx.enter_context(tc.tile_pool(name="const", bufs=1))
            z_pool = ctx.enter_context(tc.tile_pool(name="z", bufs=Z_BUFS))
            sc_pool = ctx.enter_context(tc.tile_pool(name="sc", bufs=EXT_LAG + 1))
            psum_pool = ctx.enter_context(
                tc.tile_pool(name="psum", bufs=2, space="PSUM")
            )
            acc_pool = ctx.enter_context(tc.tile_pool(name="acc", bufs=1))

            # paired operand matrices are host-assembled; the first slices
            # that gate tile 0's matmuls ship first, on both HWDGE queues
            xh = const_pool.tile([K, NH], bf16)
            yh = const_pool.tile([K, M], bf16)
            nc.sync.dma_start(xh[:, :P], xh_d[:, :P])
            nc.scalar.dma_start(yh[:, :512], yh_d[:, :512])
            nc.sync.dma_start(xh[:, P:], xh_d[:, P:])
            nc.scalar.dma_start(yh[:, 512:2048], yh_d[:, 512:2048])
            nc.sync.dma_start(yh[:, 2048:4096], yh_d[:, 2048:4096])
            nc.scalar.dma_start(yh[:, 4096:6144], yh_d[:, 4096:6144])
            nc.sync.dma_start(yh[:, 6144:], yh_d[:, 6144:])

            colacc = acc_pool.tile([P, M], f16)
            acc = acc_pool.tile([P, 36 + NB_D], f32)

            ext = []  # (acc col, scan tile, scan width) pending extraction

            def scan_into(z, lo, w, col, tag):
                # running-max scan over cols [lo, lo+2w); last col = max
                sc = sc_pool.tile([P, w], f16, tag=tag, name=f"sc_{tag}")
                nc.vector.tensor_tensor_scan(
                    sc[:], z[:, lo : lo + w], z[:, lo + w : lo + 2 * w],
                    initial=-BIG, op0=mx, op1=mx,
                )
                ext.append((col, sc, w))

            def extract():
                col, sc, w = ext.pop(0)
                nc.scalar.copy(acc[:, col : col + 1], sc[:, w - 1 : w])

            for t in range(NT):
                z = z_pool.tile([P, M], f16, tag="z")
                lhsT = xh[:, t * P : (t + 1) * P]
                for g in range(4):
                    ps = psum_pool.tile([P, GB], f32, tag="ps")
                    for j in range(4):
                        mm = g * 4 + j
                        nc.tensor.matmul(
                            ps[:, j * 512 : (j + 1) * 512],
                            lhsT,
                            yh[:, mm * 512 : (mm + 1) * 512],
                            start=True,
                            stop=True,
                        )
                    nc.scalar.activation(
                        z[:, g * GB : (g + 1) * GB], ps[:],
                        mybir.ActivationFunctionType.Copy,
                    )
                    if t == 0:
                        # tile 0: per-group quarter-scans + 4x copies so
                        # DVE starts after the FIRST drain, not the fourth
                        scan_into(
                            z, g * GB, GB // 2,
                            0 if g == 0 else 31 + g, f"q{g}",
                        )
                        nc.vector.tensor_copy(
                            colacc[:, g * GB : (g + 1) * GB],
                            z[:, g * GB : (g + 1) * GB],
                        )

                if t == NT - 1:
                    # last tile: halves, column-min TT first, so the
                    # column-fold tail starts as early as possible
                    scan_into(z, 0, GB, 31, "ha")
                    nc.vector.tensor_tensor(
                        colacc[:, : M // 2], colacc[:, : M // 2],
                        z[:, : M // 2], mx,
                    )
                    nc.vector.tensor_tensor(
                        colacc[:, M // 2 :], colacc[:, M // 2 :],
                        z[:, M // 2 :], mx,
                    )
                    scan_into(z, M // 2, GB, 35, "hb")
                elif t > 0:
                    scan_into(z, 0, M // 2, t, "sc")
                    nc.vector.tensor_tensor(colacc[:], colacc[:], z[:], mx)
                while len(ext) > EXT_LAG:
                    extract()

            while ext:
                extract()

            # --- column fold: max over the 128-partition axis of colacc ---
            # first NB_D col-blocks: PE transpose + DVE 3-D reduce; the
            # rest: gpsimd partition_all_reduce (runs in parallel on Pool)
            ident = const_pool.tile([P, P], f16)
            rowidx = const_pool.tile([P, P], f16)
            colidx = const_pool.tile([P, P], f16)
            nc.gpsimd.iota(
                rowidx[:], [[0, P]], channel_multiplier=1,
                allow_small_or_imprecise_dtypes=True,
            )
            nc.gpsimd.iota(
                colidx[:], [[1, P]], channel_multiplier=0,
                allow_small_or_imprecise_dtypes=True,
            )
            nc.vector.tensor_tensor(
                ident[:], rowidx[:], colidx[:], mybir.AluOpType.is_equal
            )

            if POOL_OPS:
                po = acc_pool.tile([P, WC], f16)
                nc.gpsimd.partition_all_reduce(
                    po[:], colacc[:, NB_D * P :],
                    channels=P, reduce_op=bass_isa.ReduceOp.max,
                )
                nc.scalar.dma_start(outc_d[:], po[0:1, :])

            nblocks = NB_D if POOL_OPS else 64
            done = 0
            while done < nblocks:
                nb = min(16, nblocks - done)
                tp = psum_pool.tile([P, nb * P], f16, tag="ps")
                for k in range(nb):
                    kk = done + k
                    nc.tensor.transpose(
                        tp[:, k * P : (k + 1) * P],
                        colacc[:, kk * P : (kk + 1) * P],
                        ident[:],
                    )
                nc.vector.tensor_reduce(
                    acc[:, 36 + done : 36 + done + nb],
                    tp.rearrange("p (k q) -> p k q", q=P),
                    axis=mybir.AxisListType.X,
                    op=mx,
                )
                done += nb
            nc.sync.dma_start(out_d[:], acc[:])

    nc.compile()
    return nc


def _get_program():
    global _PROGRAM
    if _PROGRAM is None:
        _PROGRAM = _build_program()
    return _PROGRAM


def _split3(a):
    """Split float64 array into 3 bf16 limbs: a ~= l0 + l1 + l2."""
    l0 = a.astype(ml_dtypes.bfloat16)
    r = a - l0.astype(np.float64)
    l1 = r.astype(ml_dtypes.bfloat16)
    r = r - l1.astype(np.float64)
    l2 = r.astype(ml_dtypes.bfloat16)
    return l0, l1, l2


def _unique_rows(pts, coord_scale):
    """12 unique bf16 limb rows for one side (of s = -d2).

    pts: [L, 3] f64. Rows: 3 limbs of -|p|^2, then 3 limbs of each
    coordinate scaled by coord_scale.
    """
    sq = (pts * pts).sum(1)
    rows = list(_split3(-sq))
    for i in range(3):
        rows.extend(_split3(coord_scale * pts[:, i]))
    return np.stack(rows).astype(ml_dtypes.bfloat16)


def _core_input(x, y):
    """Flat per-core input: paired x rows [24, NH] then paired y rows [24, M]."""
    xu = _unique_rows(x, coord_scale=2.0)
    yu = _unique_rows(y, coord_scale=1.0)
    ox = np.ones(x.shape[0], ml_dtypes.bfloat16)
    oy = np.ones(y.shape[0], ml_dtypes.bfloat16)
    xh = np.stack([ox if sx == "ones" else xu[sx] for sx, _ in PAIRS])
    yh = np.stack([oy if sy == "ones" else yu[sy] for _, sy in PAIRS])
    return np.concatenate([xh.ravel(), yh.ravel()])


def kernel(prediction, target):
    global LAST_RESULTS
    from concourse.bass_utils import run_bass_kernel_spmd

    nc = _get_program()

    pred = np.asarray(prediction, np.float64)
    tgt = np.asarray(target, np.float64)

    in_maps = []
    for c in range(N_CORES):
        b, h = divmod(c, 2)
        in_maps.append(
            {"xy": _core_input(pred[b, h * NH : (h + 1) * NH], tgt[b])}
        )

    res = run_bass_kernel_spmd(
        nc, in_maps, core_ids=list(range(N_CORES)), trace=TRACE
    )
    LAST_RESULTS = res

    nblocks = NB_D if POOL_OPS else 64
    cham_x = np.zeros(B)
    cham_y = np.zeros(B)
    for b in range(B):
        row = []
        cols = []
        for h in range(2):
            r = np.asarray(res.results[2 * b + h]["out"], np.float64)
            # s = -d2: rowmin_d2[p,t] = -rowmax_s; row n = t*128 + p.
            # tile 0's row max is split over 4 quarter-scans (cols 0 and
            # 32:35), tile NT-1's over 2 half-scans (cols 31 and 35)
            rmax = r[:, 0:32].copy()
            rmax[:, 0] = np.max(
                np.stack([r[:, 0], r[:, 32], r[:, 33], r[:, 34]]), axis=0
            )
            rmax[:, 31] = np.maximum(r[:, 31], r[:, 35])
            row.append(-rmax.T.ravel())
            # colmin_d2 for column m = k*128 + q (first nblocks blocks),
            # then the partition_all_reduce tail columns in plain m order
            colmin = np.empty(M)
            colmin[: nblocks * P] = -r[:, 36 : 36 + nblocks].T.ravel()
            if POOL_OPS:
                rc = np.asarray(res.results[2 * b + h]["outc"], np.float64)
                colmin[nblocks * P :] = -rc.ravel()
            cols.append(colmin)
        rowmin = np.concatenate(row)            # [N]
        colmin = np.minimum(cols[0], cols[1])   # [M]
        cham_x[b] = np.maximum(rowmin, 0.0).mean()
        cham_y[b] = np.maximum(colmin, 0.0).mean()

    return np.float32(cham_x.mean() + cham_y.mean())


# revision 37
# speedup vs baseline: 1.0319x; 1.0177x over previous
"""Chamfer distance loss on 8 Trainium2 NeuronCores.

Problem: prediction [4, 8192, 3], target [4, 8192, 3] (f32).
  d2[b,n,m] = ||pred[b,n] - tgt[b,m]||^2  (clamped at 0)
  out = mean_{b,n} min_m d2  +  mean_{b,m} min_n d2     (scalar f32)

Sharding: 8 cores = 4 batches x 2 halves of the N axis. Each core computes
its 4096 x 8192 block of the distance matrix flash-style (never
materialized in DRAM):

  - The PE produces s = -d2 via a single K=24 bf16 matmul per [128,512]
    tile using the augmented-vector trick  s = 1*(-y2) + (-x2)*1 +
    sum_i (2 x_i)*y_i, every f32 factor split into 3 bf16 limbs so
    products are exact to ~2^-26 at full bf16 PE speed. The NEGATION is
    free (host flips the limb signs) and turns both min reductions into
    MAX, which the gpsimd partition_all_reduce ISA op supports (used in
    the column-fold tail).
  - Per tile (4 PSUM groups of 2048 cols): Act drains all four groups to
    z [128,8192] f16; DVE runs the running column-max (one f16 2x
    tensor_tensor) and the row-max (one tensor_tensor_scan over the two
    row halves; its last column is the full row max).
    This Act/DVE split is FORCED: the walrus BIR verifier on this
    toolchain rejects tensor_tensor / tensor_tensor_scan / tensor_reduce
    / psum-reads on the Pool engine and 3-D tensor_tensor reads of PSUM,
    so the v6-v10 designs that balanced the drain+reductions across
    Act/DVE/Pool (~6.4us/tile in the cost model) cannot compile. DVE is
    the bound: ~8.65us/tile.
  - Scheduling details that remove the baseline's pipeline stalls
    (learned from TimelineSim traces): deps are per-engine COUNTING
    semaphores (a cross-engine wait serializes behind ALL earlier work on
    the source engine), so the row-max extraction lags TWO tiles in the
    Act stream, and input DMAs ship the tile-0-gating slices first.
  - Column-max fold over the partition axis at the end is split: PE
    transposes + DVE 3-D tensor_reduce for the first NB_D col-blocks
    (baseline-style, verifier-legal), gpsimd partition_all_reduce(max)
    for the rest, in parallel.

The paired 24-row operand matrices are host-assembled and shipped as one
flat bf16 array per core (the axon tunnel charges ~100ms per staged
array, so fewer/larger arrays win).

Host combines: row maxes (negate -> row mins), column maxes (elementwise
min of the two half-N cores after negation), relu, means.
"""

import sys

if "/opt/trn_rl_repo" not in sys.path:
    sys.path.insert(0, "/opt/trn_rl_repo")

import os as _os

import numpy as np
import ml_dtypes

B, N, M, D = 4, 8192, 8192, 3
N_CORES = 8
NH = N // 2          # rows per core (4096)
P = 128              # partitions
NT = NH // P         # n-tiles per core (32)
K = 24               # contraction rows of the split-bf16 augmented matmul
BIG = 60000.0        # > max possible d2 (~350), fits in f16
XY_LEN = K * NH + K * M  # paired x rows then paired y rows, flat

Z_BUFS = int(_os.environ.get("CHAMFER_Z_BUFS", "2"))
GB = 2048                                          # cols per PSUM group
NB_D = int(_os.environ.get("CHAMFER_NBD", "28"))  # col-blocks folded by DVE
POOL_OPS = int(_os.environ.get("CHAMFER_POOL", "1"))  # 0: all-DVE tail
EXT_LAG = 2                                        # row-max extract lag, tiles
QT = int(_os.environ.get("CHAMFER_QT", "5"))      # quarter-split lead tiles
N_EXTRA = 3 * QT + 1                               # extra row-max partials

_CFG = f"v16-{K}-{Z_BUFS}-{NB_D}-{POOL_OPS}-{QT}"


def _install_neff_cache():
    """Cache compiled NEFFs on disk keyed by a config-versioned constant.

    The stock bass_exec path recompiles (~100s of walrus) in every fresh
    process; the program here is deterministic given _CFG, so a
    config-keyed cache is safe and makes repeat runs start in seconds.
    """
    import os
    import shutil

    from concourse import bass2jax as _b2j
    from concourse import bass_utils as _bu

    if getattr(_bu, "_chamfer_neff_cache", None) == _CFG:
        return
    orig = getattr(_bu, "_chamfer_orig_compile", None) or _bu.compile_bir_kernel
    _bu._chamfer_orig_compile = orig

    def cached(bir_json, tmpdir, neff_name="file.neff"):
        key = "chamfer-" + _CFG
        cdir = os.environ.get("CHAMFER_NEFF_CACHE", "/tmp/chamfer_neff_cache")
        cpath = os.path.join(cdir, key + ".neff")
        out = os.path.join(tmpdir, neff_name)
        try:
            if os.path.exists(cpath):
                shutil.copyfile(cpath, out)
                return out
        except OSError:
            pass
        p = orig(bir_json, tmpdir, neff_name)
        try:
            os.makedirs(cdir, exist_ok=True)
            tmp = cpath + f".tmp{os.getpid()}"
            shutil.copyfile(p, tmp)
            os.replace(tmp, cpath)
        except OSError:
            pass
        return p

    _bu.compile_bir_kernel = cached
    _b2j.compile_bir_kernel = cached
    _bu._chamfer_neff_cache = _CFG


_install_neff_cache()

# Pairing of the 24 product rows: (x source, y source) where sources index
# the 12 "unique" limb rows per side, or "ones" for the constant row.
# x-unique rows: [nx2_0, nx2_1, nx2_2, a00,a01,a02, a10,a11,a12, a20,a21,a22]
# y-unique rows: [ny2_0, ny2_1, ny2_2, b00,b01,b02, b10,b11,b12, b20,b21,b22]
# where nx2/ny2 = limbs of -|p|^2, a_i* = limbs of +2*x_i, b_i* = limbs
# of y_i  (so the matmul accumulates s = -d2).
PAIRS = (
    [("ones", 0), ("ones", 1), ("ones", 2), (0, "ones"), (1, "ones"), (2, "ones")]
    + [
        (3 + 3 * i + dx, 3 + 3 * i + dy)
        for i in range(3)
        for dx, dy in ((0, 0), (0, 1), (1, 0), (0, 2), (2, 0), (1, 1))
    ]
)
assert len(PAIRS) == K

# Set by test.py.
TRACE = False
LAST_RESULTS = None

_PROGRAM = None


def _build_program():
    from concourse import bacc, tile, bass_isa
    import concourse.mybir as mybir

    f32 = mybir.dt.float32
    f16 = mybir.dt.float16
    bf16 = mybir.dt.bfloat16
    mx = mybir.AluOpType.max

    nc = bacc.Bacc(
        "TRN2",
        target_bir_lowering=False,
        debug=False,
        enable_asserts=False,
    )

    xy_d = nc.dram_tensor("xy", [XY_LEN], bf16, kind="ExternalInput").ap()
    # out[:, 0:32] row maxes of s (rowmax[p,t] = max_m s for n = t*128+p);
    # [:, 32:32+NB_D] colmax[q, k] = max_p colacc[p, 128k+q];
    # [:, 32+NB_D:] extra row-max partials: the first QT tiles' scans are
    # split per PSUM group (DVE is gated on each tile's FIRST drain
    # instead of its fourth while the Act drain pipeline ramps: Act runs
    # 7.6us/tile vs DVE 8.7us, so full-width scans would starve DVE for
    # the first ~5 tiles), and tile NT-1's scan is split in two so the
    # column-fold tail starts after its first half. 3 extras per lead
    # tile, 1 for the last tile; the host maxes them into cols 0..31.
    out_d = nc.dram_tensor(
        "out", [P, 32 + NB_D + N_EXTRA], f32, kind="ExternalOutput"
    ).ap()
    # colmax for columns [NB_D*128, 8192), via gpsimd partition_all_reduce
    WC = M - NB_D * P
    outc_d = nc.dram_tensor("outc", [1, WC], f16, kind="ExternalOutput").ap()

    xh_d = xy_d[0 : K * NH].rearrange("(k n) -> k n", k=K)
    yh_d = xy_d[K * NH :].rearrange("(k n) -> k n", k=K)

    with tile.TileContext(nc) as tc:
        from contextlib import ExitStack

        with ExitStack() as ctx:
            const_pool = ctx.enter_context(tc.tile_pool(name="const", bufs=1))
            z_pool = ctx.enter_context(tc.tile_pool(name="z", bufs=Z_BUFS))
            sc_pool = ctx.enter_context(tc.tile_pool(name="sc", bufs=EXT_LAG + 1))
            psum_pool = ctx.enter_context(
                tc.tile_pool(name="psum", bufs=2, space="PSUM")
            )
            acc_pool = ctx.enter_context(tc.tile_pool(name="acc", bufs=1))

            # paired operand matrices are host-assembled; the first slices
            # that gate tile 0's matmuls ship first, on both HWDGE queues
            xh = const_pool.tile([K, NH], bf16)
            yh = const_pool.tile([K, M], bf16)
            # tile 0 consumes yh left-to-right: g0 (DVE-drained) then
            # g1..g3 (Act); ship the columns in that order, xh's tail last
            nc.sync.dma_start(xh[:, :P], xh_d[:, :P])
            nc.scalar.dma_start(yh[:, :512], yh_d[:, :512])
            nc.sync.dma_start(yh[:, 512:2048], yh_d[:, 512:2048])
            nc.scalar.dma_start(yh[:, 2048:4096], yh_d[:, 2048:4096])
            nc.sync.dma_start(yh[:, 4096:6144], yh_d[:, 4096:6144])
            nc.scalar.dma_start(yh[:, 6144:], yh_d[:, 6144:])
            nc.sync.dma_start(xh[:, P:], xh_d[:, P:])

            colacc = acc_pool.tile([P, M], f16)
            acc = acc_pool.tile([P, 32 + NB_D + N_EXTRA], f32)

            ext = []        # (acc col, scan tile, scan width) pending
            extra_col = [32 + NB_D]  # next free extra slot

            def scan_into(z, lo, w, col, tag):
                # running-max scan over cols [lo, lo+2w); last col = max
                sc = sc_pool.tile([P, w], f16, tag=tag, name=f"sc_{tag}")
                nc.vector.tensor_tensor_scan(
                    sc[:], z[:, lo : lo + w], z[:, lo + w : lo + 2 * w],
                    initial=-BIG, op0=mx, op1=mx,
                )
                ext.append((col, sc, w))

            def extra():
                c = extra_col[0]
                extra_col[0] += 1
                return c

            def extract():
                col, sc, w = ext.pop(0)
                nc.scalar.copy(acc[:, col : col + 1], sc[:, w - 1 : w])

            z0 = acc_pool.tile([P, GB], f16)  # tile-0 g0, DVE-drained
            for t in range(NT):
                z = z_pool.tile([P, M], f16, tag="z")
                lhsT = xh[:, t * P : (t + 1) * P]
                for g in range(4):
                    ps = psum_pool.tile([P, GB], f32, tag="ps")
                    for j in range(4):
                        mm = g * 4 + j
                        nc.tensor.matmul(
                            ps[:, j * 512 : (j + 1) * 512],
                            lhsT,
                            yh[:, mm * 512 : (mm + 1) * 512],
                            start=True,
                            stop=True,
                        )
                    first = t == 0 and g == 0
                    if first:
                        # DVE drains the very first group itself, into a
                        # tile of its own (a range of z would chain Act's
                        # drains behind this copy via the same-tile-writer
                        # serialization): DVE would otherwise idle until
                        # Act's first drain lands ~3.5us later
                        nc.vector.tensor_copy(z0[:], ps[:])
                    else:
                        nc.scalar.activation(
                            z[:, g * GB : (g + 1) * GB], ps[:],
                            mybir.ActivationFunctionType.Copy,
                        )
                    if t < QT:
                        # lead tiles: per-group quarter-scans + quarter
                        # column-max ops, gated on this group's drain only
                        zt = z0 if first else z
                        scan_into(
                            zt, 0 if first else g * GB, GB // 2,
                            t if g == 0 else extra(), f"q{g}",
                        )
                        seg = colacc[:, g * GB : (g + 1) * GB]
                        zseg = zt[:, : GB] if first else z[:, g * GB : (g + 1) * GB]
                        if t == 0:
                            nc.vector.tensor_copy(seg, zseg)
                        else:
                            nc.vector.tensor_tensor(seg, seg, zseg, mx)

                if t == NT - 1:
                    # last tile: halves, column-max TT first, so the
                    # column-fold tail starts as early as possible
                    scan_into(z, 0, GB, t, "ha")
                    nc.vector.tensor_tensor(
                        colacc[:, : M // 2], colacc[:, : M // 2],
                        z[:, : M // 2], mx,
                    )
                    nc.vector.tensor_tensor(
                        colacc[:, M // 2 :], colacc[:, M // 2 :],
                        z[:, M // 2 :], mx,
                    )
                    scan_into(z, M // 2, GB, extra(), "hb")
                elif t >= QT:
                    scan_into(z, 0, M // 2, t, "sc")
                    nc.vector.tensor_tensor(colacc[:], colacc[:], z[:], mx)
                while len(ext) > EXT_LAG:
                    extract()

            while ext:
                extract()

            # --- column fold: max over the 128-partition axis of colacc ---
            # first NB_D col-blocks: PE transpose + DVE 3-D reduce; the
            # rest: gpsimd partition_all_reduce (runs in parallel on Pool)
            ident = const_pool.tile([P, P], f16)
            rowidx = const_pool.tile([P, P], f16)
            colidx = const_pool.tile([P, P], f16)
            nc.gpsimd.iota(
                rowidx[:], [[0, P]], channel_multiplier=1,
                allow_small_or_imprecise_dtypes=True,
            )
            nc.gpsimd.iota(
                colidx[:], [[1, P]], channel_multiplier=0,
                allow_small_or_imprecise_dtypes=True,
            )
            nc.vector.tensor_tensor(
                ident[:], rowidx[:], colidx[:], mybir.AluOpType.is_equal
            )

            if POOL_OPS:
                po = acc_pool.tile([P, WC], f16)
                nc.gpsimd.partition_all_reduce(
                    po[:], colacc[:, NB_D * P :],
                    channels=P, reduce_op=bass_isa.ReduceOp.max,
                )
                nc.scalar.dma_start(outc_d[:], po[0:1, :])

            nblocks = NB_D if POOL_OPS else 64
            done = 0
            while done < nblocks:
                nb = min(16, nblocks - done)
                tp = psum_pool.tile([P, nb * P], f16, tag="ps")
                for k in range(nb):
                    kk = done + k
                    nc.tensor.transpose(
                        tp[:, k * P : (k + 1) * P],
                        colacc[:, kk * P : (kk + 1) * P],
                        ident[:],
                    )
                nc.vector.tensor_reduce(
                    acc[:, 32 + done : 32 + done + nb],
                    tp.rearrange("p (k q) -> p k q", q=P),
                    axis=mybir.AxisListType.X,
                    op=mx,
                )
                done += nb
            nc.sync.dma_start(out_d[:], acc[:])

    nc.compile()
    return nc


def _get_program():
    global _PROGRAM
    if _PROGRAM is None:
        _PROGRAM = _build_program()
    return _PROGRAM


def _split3(a):
    """Split float64 array into 3 bf16 limbs: a ~= l0 + l1 + l2."""
    l0 = a.astype(ml_dtypes.bfloat16)
    r = a - l0.astype(np.float64)
    l1 = r.astype(ml_dtypes.bfloat16)
    r = r - l1.astype(np.float64)
    l2 = r.astype(ml_dtypes.bfloat16)
    return l0, l1, l2


def _unique_rows(pts, coord_scale):
    """12 unique bf16 limb rows for one side (of s = -d2).

    pts: [L, 3] f64. Rows: 3 limbs of -|p|^2, then 3 limbs of each
    coordinate scaled by coord_scale.
    """
    sq = (pts * pts).sum(1)
    rows = list(_split3(-sq))
    for i in range(3):
        rows.extend(_split3(coord_scale * pts[:, i]))
    return np.stack(rows).astype(ml_dtypes.bfloat16)


def _core_input(x, y):
    """Flat per-core input: paired x rows [24, NH] then paired y rows [24, M]."""
    xu = _unique_rows(x, coord_scale=2.0)
    yu = _unique_rows(y, coord_scale=1.0)
    ox = np.ones(x.shape[0], ml_dtypes.bfloat16)
    oy = np.ones(y.shape[0], ml_dtypes.bfloat16)
    xh = np.stack([ox if sx == "ones" else xu[sx] for sx, _ in PAIRS])
    yh = np.stack([oy if sy == "ones" else yu[sy] for _, sy in PAIRS])
    return np.concatenate([xh.ravel(), yh.ravel()])


def kernel(prediction, target):
    global LAST_RESULTS
    from concourse.bass_utils import run_bass_kernel_spmd

    nc = _get_program()

    pred = np.asarray(prediction, np.float64)
    tgt = np.asarray(target, np.float64)

    in_maps = []
    for c in range(N_CORES):
        b, h = divmod(c, 2)
        in_maps.append(
            {"xy": _core_input(pred[b, h * NH : (h + 1) * NH], tgt[b])}
        )

    res = run_bass_kernel_spmd(
        nc, in_maps, core_ids=list(range(N_CORES)), trace=TRACE
    )
    LAST_RESULTS = res

    nblocks = NB_D if POOL_OPS else 64
    cham_x = np.zeros(B)
    cham_y = np.zeros(B)
    for b in range(B):
        row = []
        cols = []
        for h in range(2):
            r = np.asarray(res.results[2 * b + h]["out"], np.float64)
            # s = -d2: rowmin_d2[p,t] = -rowmax_s; row n = t*128 + p.
            # the first QT tiles' row maxes are split over 4 quarter-scans
            # each (col t + 3 extras), tile NT-1's over 2 half-scans
            rmax = r[:, 0:32].copy()
            e0 = 32 + nblocks
            for t in range(QT):
                ex = r[:, e0 + 3 * t : e0 + 3 * t + 3]
                rmax[:, t] = np.maximum(rmax[:, t], ex.max(1))
            rmax[:, 31] = np.maximum(rmax[:, 31], r[:, e0 + 3 * QT])
            row.append(-rmax.T.ravel())
            # colmin_d2 for column m = k*128 + q (first nblocks blocks),
            # then the partition_all_reduce tail columns in plain m order
            colmin = np.empty(M)
            colmin[: nblocks * P] = -r[:, 32 : 32 + nblocks].T.ravel()
            if POOL_OPS:
                rc = np.asarray(res.results[2 * b + h]["outc"], np.float64)
                colmin[nblocks * P :] = -rc.ravel()
            cols.append(colmin)
        rowmin = np.concatenate(row)            # [N]
        colmin = np.minimum(cols[0], cols[1])   # [M]
        cham_x[b] = np.maximum(rowmin, 0.0).mean()
        cham_y[b] = np.maximum(colmin, 0.0).mean()

    return np.float32(cham_x.mean() + cham_y.mean())
